# revision 2
# baseline (speedup 1.0000x reference)
"""Single-head dot-product attention on 8 NeuronCores (Trainium2, Bass/Tile).

v2: PE runs matmuls ONLY (no PE transposes), pools hoisted out of the rep
loop so consecutive reps pipeline with no global barrier, DMA issues split
across queues (inputs + xbar-transposes on SP, outputs on DVE).

Per core (data-parallel over batch of 8):
    q = x @ Wq; k = x @ Wk; v = x @ Wv          x: [2048, 768], W*: [768, 768]
    out = softmax(q @ k.T / sqrt(768)) @ v

Formulation (identical numerics to v1):
  - Fuse q @ k.T = x (Wq Wk^T) x^T, so only t = x @ Wqk is projected.
  - scoresT[k, q] = sum_d xT[d, k] * tT[d, q]   (k on partitions)
  - expT = exp(scoresT / sqrt(768))             (no max subtraction)
  - out_ext[q, 0:769] = sum_k expT[k, q] * [v | 1][k]
  - out = out_ext[:, :768] * (1 / out_ext[:, 768])
  - All transposes (x, Wq, Wk) via DMA xbar transpose (16x128 tiles), not PE.
  - Matmul inputs bf16 (fp32 PSUM accumulation).
"""

import numpy as np

P = 128
S = 2048  # sequence length per core
D = 768   # d_model == q/k/v size
SB = S // P   # 16 s-blocks
DB = D // P   # 6 d-blocks
QSB = 256     # q-superblock (PSUM-bank limited)
NQSB = S // QSB
SCALE = 1.0 / float(np.sqrt(768.0))
N_CORES = 8

_CACHE = {}


def _build_program(reps=1):
    import concourse.bacc as bacc
    import concourse.mybir as mybir
    import concourse.tile as tile

    f32 = mybir.dt.float32
    bf16 = mybir.dt.bfloat16
    EXP = mybir.ActivationFunctionType.Exp

    nc = bacc.Bacc("TRN2", target_bir_lowering=False, debug=False,
                   num_devices=N_CORES)
    x_dram = nc.dram_tensor("x", [S, D], f32, kind="ExternalInput")
    wq_dram = nc.dram_tensor("wq", [D, D], f32, kind="ExternalInput")
    wk_dram = nc.dram_tensor("wk", [D, D], f32, kind="ExternalInput")
    wv_dram = nc.dram_tensor("wv", [D, D], f32, kind="ExternalInput")
    y_dram = nc.dram_tensor("y", [S, D], f32, kind="ExternalOutput")

    with tile.TileContext(nc) as tc:
        # Pools live for the whole program (all reps) so rep N+1's DMAs and
        # PE work overlap rep N's tail via per-tile WAR deps, not barriers.
        persist = tc.alloc_tile_pool(name="persist", bufs=1)
        xtp = tc.alloc_tile_pool(name="xtp", bufs=2)
        wfp = tc.alloc_tile_pool(name="wfp", bufs=3)
        wbp = tc.alloc_tile_pool(name="wbp", bufs=3)
        xfp = tc.alloc_tile_pool(name="xfp", bufs=6)
        xbp = tc.alloc_tile_pool(name="xbp", bufs=4)
        exp_pool = tc.alloc_tile_pool(name="exp", bufs=3)
        yout = tc.alloc_tile_pool(name="yout", bufs=4)
        pa = tc.alloc_tile_pool(name="pa", bufs=3, space="PSUM")
        pb = tc.alloc_tile_pool(name="pb", bufs=3, space="PSUM")
        scp = tc.alloc_tile_pool(name="scp", bufs=2, space="PSUM")

        for _rep in range(reps):
            # Persistent per-rep tensors (tags stable across reps -> rotate).
            # xT_all[p, sb, db, s] = x[sb*128+s, db*128+p]  (bufs=2 so next
            # rep's transposes land while this rep still reads buffer A).
            xT_all = xtp.tile([P, SB, DB, P], bf16, tag="xT", name="xT_all")
            tT = [persist.tile([P, S], bf16, tag=f"tT{i}", name=f"tT{i}")
                  for i in range(DB)]
            vE = [persist.tile([P, 776], bf16, tag=f"vE{i}", name=f"vE{i}")
                  for i in range(SB)]
            # wqT_all[p, dblk, eb, d] = Wq[dblk*128+d, eb*128+p]
            wqT_all = persist.tile([P, DB, DB, P], bf16, tag="wqT",
                                   name="wqT_all")
            wkT_all = persist.tile([P, DB, DB, P], bf16, tag="wkT",
                                   name="wkT_all")
            wqk = [persist.tile([P, D], bf16, tag=f"wqk{i}", name=f"wqk{i}")
                   for i in range(DB)]
            wv_all = persist.tile([P, DB, D], bf16, tag="wv", name="wv_all")

            # ---- input DMAs (SP queue), casts (ACT), transposes (SP) ----
            # Issue order: first x tiles (v_proj gates on them), then w.
            xf = []
            for sb in range(4):
                t = xfp.tile([P, D], f32, tag="xf", name=f"xf{sb}")
                nc.sync.dma_start(t, x_dram[sb * P:(sb + 1) * P, :])
                xf.append(t)
            wqf, wkf, wvf = [], [], []
            for db in range(DB):
                t = wfp.tile([P, D], f32, tag="wvf", name=f"wvf{db}")
                nc.sync.dma_start(t, wv_dram[db * P:(db + 1) * P, :])
                wvf.append(t)
            for db in range(DB):
                t = wfp.tile([P, D], f32, tag="wqf", name=f"wqf{db}")
                nc.sync.dma_start(t, wq_dram[db * P:(db + 1) * P, :])
                wqf.append(t)
                t = wfp.tile([P, D], f32, tag="wkf", name=f"wkf{db}")
                nc.sync.dma_start(t, wk_dram[db * P:(db + 1) * P, :])
                wkf.append(t)
            for sb in range(4, SB):
                t = xfp.tile([P, D], f32, tag="xf", name=f"xf{sb}")
                nc.sync.dma_start(t, x_dram[sb * P:(sb + 1) * P, :])
                xf.append(t)

            # ACT casts + SP xbar transposes, in consumer-priority order:
            # x[0:4] (first v_projs), wv, wq/wk (wqk), x[4:].
            def x_prep(sb):
                xb = xbp.tile([P, D], bf16, tag="xb", name=f"xb{sb}")
                nc.scalar.copy(xb, xf[sb])
                nc.sync.dma_start_transpose(xT_all[:, sb], xb)

            for sb in range(4):
                x_prep(sb)
            for db in range(DB):
                nc.scalar.copy(wv_all[:, db, :], wvf[db])
            for db in range(DB):
                for src, dstT, tag in ((wqf, wqT_all, "wqb"),
                                       (wkf, wkT_all, "wkb")):
                    wb = wbp.tile([P, D], bf16, tag=tag, name=f"{tag}{db}")
                    nc.scalar.copy(wb, src[db])
                    nc.sync.dma_start_transpose(dstT[:, db], wb)
            for sb in range(4, SB):
                x_prep(sb)

            # ---- projections (PE) ----
            # v[s, e] = sum_d x[s, d] Wv[d, e]; vE = [v | 1]
            def v_proj(sb):
                pv_a = pa.tile([P, 512], f32, tag="a512", name="pv_a")
                pv_b = pb.tile([P, 257], f32, tag="b257", name="pv_b")
                for db in range(DB):
                    lhs = xT_all[:, sb, db, :]
                    nc.tensor.matmul(pv_a, lhs, wv_all[:, db, 0:512],
                                     start=(db == 0), stop=(db == DB - 1))
                    nc.tensor.matmul(pv_b[:, 0:256], lhs,
                                     wv_all[:, db, 512:768],
                                     start=(db == 0), stop=(db == DB - 1))
                nc.vector.tensor_copy(vE[sb][:, 0:512], pv_a)
                nc.vector.tensor_copy(vE[sb][:, 512:768], pv_b[:, 0:256])
                nc.gpsimd.memset(vE[sb][:, 768:769], 1.0)

            # Wqk[d1, d2] = sum_e Wq[d1, e] Wk[d2, e]
            def wqk_block(d1):
                qk_a = pa.tile([P, 512], f32, tag="a512", name="qk_a")
                qk_b = pb.tile([P, 257], f32, tag="b257", name="qk_b")
                for eb in range(DB):
                    lhs = wqT_all[:, d1, eb, :]
                    nc.tensor.matmul(qk_a, lhs, wkT_all[:, 0:4, eb, :],
                                     start=(eb == 0), stop=(eb == DB - 1))
                    nc.tensor.matmul(qk_b[:, 0:256], lhs,
                                     wkT_all[:, 4:6, eb, :],
                                     start=(eb == 0), stop=(eb == DB - 1))
                nc.vector.tensor_copy(wqk[d1][:, 0:512], qk_a)
                nc.vector.tensor_copy(wqk[d1][:, 512:768], qk_b[:, 0:256])

            # tT[d2, s] = sum_d1 Wqk[d1, d2] xT[d1, s] for 512-wide chunk g
            def t_proj(g):
                for d2 in range(DB):
                    pj = pa.tile([P, 512], f32, tag="a512", name="pj")
                    for db in range(DB):
                        nc.tensor.matmul(
                            pj, wqk[db][:, d2 * P:(d2 + 1) * P],
                            xT_all[:, 4 * g:4 * (g + 1), db, :],
                            start=(db == 0), stop=(db == DB - 1))
                    nc.vector.tensor_copy(
                        tT[d2][:, g * 512:(g + 1) * 512], pj)

            for sb in range(4):
                v_proj(sb)
            for d1 in range(DB):
                wqk_block(d1)
            t_proj(0)
            for sb in range(4, 8):
                v_proj(sb)
            t_proj(1)
            for sb in range(8, 12):
                v_proj(sb)
            t_proj(2)
            for sb in range(12, SB):
                v_proj(sb)
            t_proj(3)

            # ---- attention ----
            for qsb in range(NQSB):
                q0 = qsb * QSB
                oa = [pa.tile([P, 512], f32, tag="a512", name=f"oa{qi}")
                      for qi in range(QSB // P)]
                ob = [pb.tile([P, 257], f32, tag="b257", name=f"ob{qi}")
                      for qi in range(QSB // P)]

                def out_mms(ki, ex):
                    for qi in range(QSB // P):
                        lhs = ex[:, qi * P:(qi + 1) * P]
                        nc.tensor.matmul(oa[qi], lhs, vE[ki][:, 0:512],
                                         start=(ki == 0), stop=(ki == SB - 1))
                        nc.tensor.matmul(ob[qi], lhs, vE[ki][:, 512:769],
                                         start=(ki == 0), stop=(ki == SB - 1))

                prev = None
                for ki in range(SB):
                    sc = scp.tile([P, QSB], f32, tag="sc", name="sc")
                    for db in range(DB):
                        nc.tensor.matmul(
                            sc, xT_all[:, ki, db, :],
                            tT[db][:, q0:q0 + QSB],
                            start=(db == 0), stop=(db == DB - 1))
                    ex = exp_pool.tile([P, QSB], bf16, tag="ex", name="ex")
                    nc.scalar.activation(ex, sc, EXP, scale=SCALE)
                    if prev is not None:
                        out_mms(*prev)
                    prev = (ki, ex)
                out_mms(*prev)

                for qi in range(QSB // P):
                    den = yout.tile([P, 1], f32, tag="den", name="den")
                    nc.vector.reciprocal(den, ob[qi][:, 256:257])
                    yt = yout.tile([P, D], f32, tag="yt", name="yt")
                    nc.vector.tensor_scalar_mul(yt[:, 0:512], oa[qi], den)
                    nc.vector.tensor_scalar_mul(
                        yt[:, 512:768], ob[qi][:, 0:256], den)
                    r0 = q0 + qi * P
                    nc.gpsimd.dma_start(y_dram[r0:r0 + P, :], yt)

        for pool in reversed((persist, xtp, wfp, wbp, xfp, xbp, exp_pool,
                              yout, pa, pb, scp)):
            pool.release()

    nc.compile()
    return nc


def _get_program():
    if "nc" not in _CACHE:
        _CACHE["nc"] = _build_program()
    return _CACHE["nc"]


def _get_runner():
    """Build the program once and wrap it in a cached sharded jit callable."""
    if "runner" in _CACHE:
        return _CACHE["runner"]

    import jax
    from jax.experimental.shard_map import shard_map
    from jax.sharding import Mesh, PartitionSpec

    import concourse.mybir as mybir
    from concourse.bass2jax import (
        _bass_exec_p,
        install_neuronx_cc_hook,
        partition_id_tensor,
    )

    nc = _get_program()
    install_neuronx_cc_hook()

    partition_name = (nc.partition_id_tensor.name
                      if nc.partition_id_tensor else None)
    in_names, out_names, out_avals, zero_shapes = [], [], [], []
    for alloc in nc.m.functions[0].allocations:
        if not isinstance(alloc, mybir.MemoryLocationSet):
            continue
        name = alloc.memorylocations[0].name
        if alloc.kind == "ExternalInput":
            if name != partition_name:
                in_names.append(name)
        elif alloc.kind == "ExternalOutput":
            out_names.append(name)
            shape = tuple(alloc.tensor_shape)
            dtype = mybir.dt.np(alloc.dtype)
            out_avals.append(jax.core.ShapedArray(shape, dtype))
            zero_shapes.append((shape, dtype))
    n_params = len(in_names)
    all_names = list(in_names) + list(out_names)
    if partition_name is not None:
        all_names.append(partition_name)

    def _body(*args):
        operands = list(args)
        if partition_name is not None:
            operands.append(partition_id_tensor())
        outs = _bass_exec_p.bind(
            *operands,
            out_avals=tuple(out_avals),
            in_names=tuple(all_names),
            out_names=tuple(out_names),
            lowering_input_output_aliases=(),
            sim_require_finite=True,
            sim_require_nnan=True,
            nc=nc,
        )
        return tuple(outs)

    devices = jax.devices()[:N_CORES]
    mesh = Mesh(np.asarray(devices), ("core",))
    n_outs = len(out_names)
    sharded = jax.jit(
        shard_map(_body, mesh=mesh,
                  in_specs=(PartitionSpec("core"),) * (n_params + n_outs),
                  out_specs=(PartitionSpec("core"),) * n_outs,
                  check_rep=False),
        donate_argnums=tuple(range(n_params, n_params + n_outs)),
        keep_unused=True,
    )
    _CACHE["runner"] = (sharded, in_names, zero_shapes)
    return _CACHE["runner"]


def kernel(**inputs):
    sharded, in_names, zero_shapes = _get_runner()

    x = np.ascontiguousarray(np.asarray(inputs["inputs"], dtype=np.float32))
    wq = np.ascontiguousarray(np.asarray(inputs["W_query"], dtype=np.float32))
    wk = np.ascontiguousarray(np.asarray(inputs["W_key"], dtype=np.float32))
    wv = np.ascontiguousarray(np.asarray(inputs["W_value"], dtype=np.float32))
    per_core = {
        "x": [x[b] for b in range(N_CORES)],
        "wq": [wq] * N_CORES,
        "wk": [wk] * N_CORES,
        "wv": [wv] * N_CORES,
    }
    concat_in = [np.concatenate(per_core[nm], axis=0) for nm in in_names]
    concat_zeros = [np.zeros((N_CORES * sh[0], *sh[1:]), dt)
                    for sh, dt in zero_shapes]
    outs = sharded(*concat_in, *concat_zeros)
    y = np.asarray(outs[0]).reshape(N_CORES, S, D)
    return y


# revision 3
# speedup vs baseline: 1.2577x; 1.2577x over previous
"""Single-head dot-product attention on 8 NeuronCores (Trainium2, Bass/Tile).

v2a: like v2 (hoisted pools, cross-rep pipelining, split DMA ordering) but
with PE transposes (bf16) instead of DMA xbar transposes, and y output DMAs
on the SP queue. PE cost: matmuls + 168 bf16 128-col transposes.
"""

import numpy as np

P = 128
S = 2048
D = 768
SB = S // P   # 16
DB = D // P   # 6
QSB = 256
NQSB = S // QSB
SCALE = 1.0 / float(np.sqrt(768.0))
N_CORES = 8

_CACHE = {}


def _build_program(reps=1):
    import concourse.bacc as bacc
    import concourse.mybir as mybir
    import concourse.tile as tile
    from concourse.masks import make_identity

    f32 = mybir.dt.float32
    bf16 = mybir.dt.bfloat16
    EXP = mybir.ActivationFunctionType.Exp

    nc = bacc.Bacc("TRN2", target_bir_lowering=False, debug=False,
                   num_devices=N_CORES)
    x_dram = nc.dram_tensor("x", [S, D], f32, kind="ExternalInput")
    wq_dram = nc.dram_tensor("wq", [D, D], f32, kind="ExternalInput")
    wk_dram = nc.dram_tensor("wk", [D, D], f32, kind="ExternalInput")
    wv_dram = nc.dram_tensor("wv", [D, D], f32, kind="ExternalInput")
    y_dram = nc.dram_tensor("y", [S, D], f32, kind="ExternalOutput")

    with tile.TileContext(nc) as tc:
        persist = tc.alloc_tile_pool(name="persist", bufs=1)
        xtp = tc.alloc_tile_pool(name="xtp", bufs=2)
        wfp = tc.alloc_tile_pool(name="wfp", bufs=3)
        wbp = tc.alloc_tile_pool(name="wbp", bufs=3)
        xfp = tc.alloc_tile_pool(name="xfp", bufs=6)
        xbp = tc.alloc_tile_pool(name="xbp", bufs=4)
        exp_pool = tc.alloc_tile_pool(name="exp", bufs=3)
        yout = tc.alloc_tile_pool(name="yout", bufs=4)
        pa = tc.alloc_tile_pool(name="pa", bufs=3, space="PSUM")
        pb = tc.alloc_tile_pool(name="pb", bufs=2, space="PSUM")
        scp = tc.alloc_tile_pool(name="scp", bufs=3, space="PSUM")

        for _rep in range(reps):
            ident = persist.tile([P, P], bf16, tag="ident", name="ident")
            make_identity(nc, ident)

            xT_all = xtp.tile([P, SB, DB, P], bf16, tag="xT", name="xT_all")
            tT = [persist.tile([P, S], bf16, tag=f"tT{i}", name=f"tT{i}")
                  for i in range(DB)]
            vE = [persist.tile([P, 776], bf16, tag=f"vE{i}", name=f"vE{i}")
                  for i in range(SB)]
            wqT_all = persist.tile([P, DB, DB, P], bf16, tag="wqT",
                                   name="wqT_all")
            wkT_all = persist.tile([P, DB, DB, P], bf16, tag="wkT",
                                   name="wkT_all")
            wqk = [persist.tile([P, D], bf16, tag=f"wqk{i}", name=f"wqk{i}")
                   for i in range(DB)]
            wv_all = persist.tile([P, DB, D], bf16, tag="wv", name="wv_all")

            # ---- input DMAs (SP) ----
            xf = []
            for sb in range(4):
                t = xfp.tile([P, D], f32, tag="xf", name=f"xf{sb}")
                nc.sync.dma_start(t, x_dram[sb * P:(sb + 1) * P, :])
                xf.append(t)
            wqf, wkf, wvf = [], [], []
            for db in range(DB):
                t = wfp.tile([P, D], f32, tag="wvf", name=f"wvf{db}")
                nc.sync.dma_start(t, wv_dram[db * P:(db + 1) * P, :])
                wvf.append(t)
            for db in range(DB):
                t = wfp.tile([P, D], f32, tag="wqf", name=f"wqf{db}")
                nc.sync.dma_start(t, wq_dram[db * P:(db + 1) * P, :])
                wqf.append(t)
                t = wfp.tile([P, D], f32, tag="wkf", name=f"wkf{db}")
                nc.sync.dma_start(t, wk_dram[db * P:(db + 1) * P, :])
                wkf.append(t)
            for sb in range(4, SB):
                t = xfp.tile([P, D], f32, tag="xf", name=f"xf{sb}")
                nc.sync.dma_start(t, x_dram[sb * P:(sb + 1) * P, :])
                xf.append(t)

            # ---- casts (ACT) + PE transposes (into sc-tag PSUM, bf16) ----
            def x_prep(sb):
                xb = xbp.tile([P, D], bf16, tag="xb", name=f"xb{sb}")
                nc.scalar.copy(xb, xf[sb])
                pt = scp.tile([P, D], bf16, tag="sc", name="pt")
                for db in range(DB):
                    nc.tensor.transpose(
                        pt[:, db * P:(db + 1) * P],
                        xb[:, db * P:(db + 1) * P], ident)
                nc.vector.tensor_copy(
                    xT_all[:, sb],
                    pt.rearrange("p (a b) -> p a b", a=DB))

            def w_prep(db, src, dstT, tag):
                wb = wbp.tile([P, D], bf16, tag=tag, name=f"{tag}{db}")
                nc.scalar.copy(wb, src[db])
                ptw = scp.tile([P, D], bf16, tag="sc", name="ptw")
                for eb in range(DB):
                    nc.tensor.transpose(
                        ptw[:, eb * P:(eb + 1) * P],
                        wb[:, eb * P:(eb + 1) * P], ident)
                nc.vector.tensor_copy(
                    dstT[:, db],
                    ptw.rearrange("p (a b) -> p a b", a=DB))

            for sb in range(4):
                x_prep(sb)
            for db in range(DB):
                nc.scalar.copy(wv_all[:, db, :], wvf[db])
            for db in range(DB):
                w_prep(db, wqf, wqT_all, "wqb")
                w_prep(db, wkf, wkT_all, "wkb")
            for sb in range(4, SB):
                x_prep(sb)

            # ---- projections (PE) ----
            def v_proj(sb):
                pv_a = pa.tile([P, 512], f32, tag="a512", name="pv_a")
                pv_b = pb.tile([P, 257], f32, tag="b257", name="pv_b")
                for db in range(DB):
                    lhs = xT_all[:, sb, db, :]
                    nc.tensor.matmul(pv_a, lhs, wv_all[:, db, 0:512],
                                     start=(db == 0), stop=(db == DB - 1))
                    nc.tensor.matmul(pv_b[:, 0:256], lhs,
                                     wv_all[:, db, 512:768],
                                     start=(db == 0), stop=(db == DB - 1))
                nc.vector.tensor_copy(vE[sb][:, 0:512], pv_a)
                nc.vector.tensor_copy(vE[sb][:, 512:768], pv_b[:, 0:256])
                nc.gpsimd.memset(vE[sb][:, 768:769], 1.0)

            def wqk_block(d1):
                qk_a = pa.tile([P, 512], f32, tag="a512", name="qk_a")
                qk_b = pb.tile([P, 257], f32, tag="b257", name="qk_b")
                for eb in range(DB):
                    lhs = wqT_all[:, d1, eb, :]
                    nc.tensor.matmul(qk_a, lhs, wkT_all[:, 0:4, eb, :],
                                     start=(eb == 0), stop=(eb == DB - 1))
                    nc.tensor.matmul(qk_b[:, 0:256], lhs,
                                     wkT_all[:, 4:6, eb, :],
                                     start=(eb == 0), stop=(eb == DB - 1))
                nc.vector.tensor_copy(wqk[d1][:, 0:512], qk_a)
                nc.vector.tensor_copy(wqk[d1][:, 512:768], qk_b[:, 0:256])

            def t_proj(g):
                for d2 in range(DB):
                    pj = pa.tile([P, 512], f32, tag="a512", name="pj")
                    for db in range(DB):
                        nc.tensor.matmul(
                            pj, wqk[db][:, d2 * P:(d2 + 1) * P],
                            xT_all[:, 4 * g:4 * (g + 1), db, :],
                            start=(db == 0), stop=(db == DB - 1))
                    nc.vector.tensor_copy(
                        tT[d2][:, g * 512:(g + 1) * 512], pj)

            for sb in range(4):
                v_proj(sb)
            for d1 in range(DB):
                wqk_block(d1)
            t_proj(0)
            for sb in range(4, 8):
                v_proj(sb)
            t_proj(1)
            for sb in range(8, 12):
                v_proj(sb)
            t_proj(2)
            for sb in range(12, SB):
                v_proj(sb)
            t_proj(3)

            # ---- attention ----
            for qsb in range(NQSB):
                q0 = qsb * QSB
                oa = [pa.tile([P, 512], f32, tag="a512", name=f"oa{qi}")
                      for qi in range(QSB // P)]
                ob = [pb.tile([P, 257], f32, tag="b257", name=f"ob{qi}")
                      for qi in range(QSB // P)]

                def out_mms(ki, ex):
                    for qi in range(QSB // P):
                        lhs = ex[:, qi * P:(qi + 1) * P]
                        nc.tensor.matmul(oa[qi], lhs, vE[ki][:, 0:512],
                                         start=(ki == 0), stop=(ki == SB - 1))
                        nc.tensor.matmul(ob[qi], lhs, vE[ki][:, 512:769],
                                         start=(ki == 0), stop=(ki == SB - 1))

                prev = None
                for ki in range(SB):
                    sc = scp.tile([P, QSB], f32, tag="sc", name="sc")
                    for db in range(DB):
                        nc.tensor.matmul(
                            sc, xT_all[:, ki, db, :],
                            tT[db][:, q0:q0 + QSB],
                            start=(db == 0), stop=(db == DB - 1))
                    ex = exp_pool.tile([P, QSB], bf16, tag="ex", name="ex")
                    nc.scalar.activation(ex, sc, EXP, scale=SCALE)
                    if prev is not None:
                        out_mms(*prev)
                    prev = (ki, ex)
                out_mms(*prev)

                for qi in range(QSB // P):
                    den = yout.tile([P, 1], f32, tag="den", name="den")
                    nc.vector.reciprocal(den, ob[qi][:, 256:257])
                    yt = yout.tile([P, D], f32, tag="yt", name="yt")
                    nc.vector.tensor_scalar_mul(yt[:, 0:512], oa[qi], den)
                    nc.vector.tensor_scalar_mul(
                        yt[:, 512:768], ob[qi][:, 0:256], den)
                    r0 = q0 + qi * P
                    nc.sync.dma_start(y_dram[r0:r0 + P, :], yt)

        for pool in reversed((persist, xtp, wfp, wbp, xfp, xbp, exp_pool,
                              yout, pa, pb, scp)):
            pool.release()

    nc.compile()
    return nc


def _get_program():
    if "nc" not in _CACHE:
        _CACHE["nc"] = _build_program()
    return _CACHE["nc"]


def _get_runner():
    if "runner" in _CACHE:
        return _CACHE["runner"]

    import jax
    from jax.experimental.shard_map import shard_map
    from jax.sharding import Mesh, PartitionSpec

    import concourse.mybir as mybir
    from concourse.bass2jax import (
        _bass_exec_p,
        install_neuronx_cc_hook,
        partition_id_tensor,
    )

    nc = _get_program()
    install_neuronx_cc_hook()

    partition_name = (nc.partition_id_tensor.name
                      if nc.partition_id_tensor else None)
    in_names, out_names, out_avals, zero_shapes = [], [], [], []
    for alloc in nc.m.functions[0].allocations:
        if not isinstance(alloc, mybir.MemoryLocationSet):
            continue
        name = alloc.memorylocations[0].name
        if alloc.kind == "ExternalInput":
            if name != partition_name:
                in_names.append(name)
        elif alloc.kind == "ExternalOutput":
            out_names.append(name)
            shape = tuple(alloc.tensor_shape)
            dtype = mybir.dt.np(alloc.dtype)
            out_avals.append(jax.core.ShapedArray(shape, dtype))
            zero_shapes.append((shape, dtype))
    n_params = len(in_names)
    all_names = list(in_names) + list(out_names)
    if partition_name is not None:
        all_names.append(partition_name)

    def _body(*args):
        operands = list(args)
        if partition_name is not None:
            operands.append(partition_id_tensor())
        outs = _bass_exec_p.bind(
            *operands,
            out_avals=tuple(out_avals),
            in_names=tuple(all_names),
            out_names=tuple(out_names),
            lowering_input_output_aliases=(),
            sim_require_finite=True,
            sim_require_nnan=True,
            nc=nc,
        )
        return tuple(outs)

    devices = jax.devices()[:N_CORES]
    mesh = Mesh(np.asarray(devices), ("core",))
    n_outs = len(out_names)
    sharded = jax.jit(
        shard_map(_body, mesh=mesh,
                  in_specs=(PartitionSpec("core"),) * (n_params + n_outs),
                  out_specs=(PartitionSpec("core"),) * n_outs,
                  check_rep=False),
        donate_argnums=tuple(range(n_params, n_params + n_outs)),
        keep_unused=True,
    )
    _CACHE["runner"] = (sharded, in_names, zero_shapes)
    return _CACHE["runner"]


def kernel(**inputs):
    sharded, in_names, zero_shapes = _get_runner()

    x = np.ascontiguousarray(np.asarray(inputs["inputs"], dtype=np.float32))
    wq = np.ascontiguousarray(np.asarray(inputs["W_query"], dtype=np.float32))
    wk = np.ascontiguousarray(np.asarray(inputs["W_key"], dtype=np.float32))
    wv = np.ascontiguousarray(np.asarray(inputs["W_value"], dtype=np.float32))
    per_core = {
        "x": [x[b] for b in range(N_CORES)],
        "wq": [wq] * N_CORES,
        "wk": [wk] * N_CORES,
        "wv": [wv] * N_CORES,
    }
    concat_in = [np.concatenate(per_core[nm], axis=0) for nm in in_names]
    concat_zeros = [np.zeros((N_CORES * sh[0], *sh[1:]), dt)
                    for sh, dt in zero_shapes]
    outs = sharded(*concat_in, *concat_zeros)
    y = np.asarray(outs[0]).reshape(N_CORES, S, D)
    return y
